# revision 15
# baseline (speedup 1.0000x reference)
"""CompressedKVCache kernel for Trainium2 (8 NeuronCores, head-sharded).

Per (b, h) head: quantize k/v rows to int4 (per-row min/max affine),
then return the dequantized cache prefix [0, start+L): rows [0, start)
decoded from the packed uint8 cache inputs, rows [start, start+L)
quantize->dequantized entirely on-chip.

Sharding: H=32 heads split across 8 cores (4 heads each); fully
independent per head, no cross-core communication.

v4 schedule: k and v of one (b, hh) form a GROUP (8 groups/core).
Shared x2/pkt2 tiles let the min/max reduces, the stats chain and the
nibble unpack run paired (one instruction covers both tensors), cutting
DVE instruction overhead.

Staleness discipline (the v2 lesson): every cross-engine dependency is
satisfied >= 1 block before its consumer issues, so no engine's wait
queue ever backs up and the in-order SP DMA queue never head-of-line
blocks the input loads. Per block g:
  SP   - loads of group g+3, then stores of group g-2 (all inputs to
         these DMAs were ready by the end of block g-1)
  DVE  - reduces/stats/unpack for g+1 (lookahead), then deq share of g-1
  ACT  - deq share of g-1 first, then quant k(g), quant v(g)
  Pool - prefix dequant of g (lohi from block g-1), deq share of g-1
Tail: from block 5 on SP has no loads left, so stores turn eager
(freshest-ready), leaving only group 7's dequant + 2 MiB of oq stores
after the last block.

Engine split per group (96 affine chunks of [128,128]):
  ACT 32 quant + 10 deq (~16.6us), Pool 32 prefix (~14.8us, the per-chunk
  slowest engine gets no deq in steady state), DVE reduces+stats+unpack
  (~12us) + 22 deq (~16.8us). The last two groups 3-way split their deq
  so the epilogue drains fast. All three engines RNE-round on f32->u8
  (probe-verified), but quant stays on ACT (fused scale/bias form).

Measured lessons (HW traces): any "just-stale" tile-ring reuse or
same-block cross-engine dependency adds ~100ns/instruction of semaphore
wait and can gate a consumer on the CUMULATIVE DMA semaphore (waiting for
unrelated later stores). Ring depths of 3+ and one-block staleness
everywhere are worth more than deeper prefetch or in-place-output tricks
(v5/v6 regressions).
"""

import sys

sys.path.insert(0, "/opt/trn_rl_repo")

import numpy as np
from concourse import bass, mybir
from concourse import tile
from concourse.bass_utils import run_bass_kernel_spmd

F32 = mybir.dt.float32
U8 = mybir.dt.uint8
U32 = mybir.dt.uint32
Alu = mybir.AluOpType
Act = mybir.ActivationFunctionType
AX = mybir.AxisListType
INV15 = float(np.float32(1.0 / 15.0))

B, H, L, D = 2, 32, 2048, 128
MAX_SEQ = 8192
N_CORES = 8
HC = H // N_CORES   # heads per core
CQ = L // 128       # quant row-chunks per head (16)
NG = B * HC         # groups per core (8); one group = k+v of one (b,hh)

# Per-unit deq chunk split (of CQ=16): (ACT, Pool) -- DVE gets the rest.
# Pool is the busiest engine (32 prefix chunks/group at ~460ns), so in
# steady state it gets no deq work; the last groups 3-way split so the
# epilogue dequant of group 7 finishes fast on all engines.
DEQ_SPLIT = {6: (0, 0), 7: (6, 4)}
DEQ_DEFAULT = (4, 0)
# Prefix chunks on Pool per unit for the tail group (DVE takes the rest)
PRE_SPLIT = {7: 12}

PREFETCH = 3  # groups of input lookahead


def _split_multiwait(nc):
    """This container's walrus accepts only ONE sync-wait per instruction;
    Tile's tail drain (and occasionally other insts) carry several. Split
    extras into single-wait EventSemaphore insts inserted just before."""
    for fn in nc.m.functions:
        for blk in fn.blocks:
            out = []
            for ins in blk.instructions:
                si = ins.sync_info
                if si is not None and si.on_wait is not None and len(si.on_wait) > 1:
                    waits = list(si.on_wait)
                    for j, w in enumerate(waits[:-1]):
                        out.append(mybir.InstEventSemaphore(
                            name=f"{ins.name}_sw{j}", ins=[], outs=[],
                            engine=ins.engine,
                            sync_info=mybir.SyncInfo(on_wait=[w], on_update=[])))
                    si.on_wait = [waits[-1]]
                    ins.sync_info = si
                out.append(ins)
            blk.instructions = out


def _build(start_pos: int, split_mw: bool = True):
    """Per core: xk/xv (B,HC,L,D) f32, prefix packed caches (B,HC,S,64) u8
    and prefix scale/zero rows (B,HC,S) f32 -> ok/ov (B,HC,S+L,D) f32."""
    S = start_pos
    E = S + L
    CP = S // 128  # prefix row-chunks per head
    assert L % 128 == 0 and S % 128 == 0 and E <= MAX_SEQ

    nc = bass.Bass(trn_type="TRN2")

    ins_q, ins_p, ins_sc, ins_zp, outs = {}, {}, {}, {}, {}
    for t in ("k", "v"):
        ins_q[t] = nc.dram_tensor(f"x{t}", [B, HC, L, D], F32, kind="ExternalInput")
        if S:
            ins_p[t] = nc.dram_tensor(f"p{t}", [B, HC, S, D // 2], U8, kind="ExternalInput")
            ins_sc[t] = nc.dram_tensor(f"sc{t}", [B, HC, S], F32, kind="ExternalInput")
            ins_zp[t] = nc.dram_tensor(f"zp{t}", [B, HC, S], F32, kind="ExternalInput")
        outs[t] = nc.dram_tensor(f"o{t}", [B, HC, E, D], F32, kind="ExternalOutput")

    groups = [(b, hh) for b in range(B) for hh in range(HC)]
    PC = B * HC * CP  # hoisted scale/zero columns per tensor

    with tile.TileContext(nc) as tc:
        with tc.tile_pool(name="xp", bufs=PREFETCH + 1) as xp, \
             tc.tile_pool(name="pktp", bufs=PREFETCH + 1) as pktp, \
             tc.tile_pool(name="lohip", bufs=3) as lohip, \
             tc.tile_pool(name="qp", bufs=3) as qp, \
             tc.tile_pool(name="oqp", bufs=3) as oqp, \
             tc.tile_pool(name="oprep", bufs=3) as oprep, \
             tc.tile_pool(name="small", bufs=4) as small, \
             tc.tile_pool(name="persist", bufs=1) as persist:

            st = [None] * NG  # live tiles per group

            def load_group(g):
                b, hh = groups[g]
                x2 = xp.tile([128, 2, CQ, D], F32, tag="x2")
                for kv, t in enumerate(("k", "v")):
                    nc.sync.dma_start(
                        out=x2[:, kv, :, :],
                        in_=ins_q[t][b, hh, :, :].rearrange("(p c) d -> p c d", p=128))
                pkt2 = None
                if S:
                    pkt2 = pktp.tile([128, 2, CP, D // 2], U8, tag="pkt2")
                    for kv, t in enumerate(("k", "v")):
                        nc.sync.dma_start(
                            out=pkt2[:, kv, :, :],
                            in_=ins_p[t][b, hh, :, :].rearrange("(p c) d -> p c d", p=128))
                st[g] = {"x2": x2, "pkt2": pkt2}

            def dve_lookahead(g, split_first=False):
                """Reduces + stats (+ unpack) for group g on DVE."""
                d = st[g]
                x2 = d["x2"]
                mx2 = small.tile([128, 2, CQ], F32, tag="mx2")
                mn2 = small.tile([128, 2, CQ], F32, tag="mn2")
                scl2 = small.tile([128, 2, CQ], F32, tag="scl2")
                rcp2 = small.tile([128, 2, CQ], F32, tag="rcp2")
                zero2 = small.tile([128, 2, CQ], F32, tag="zero2")

                def stats_chain(sl):
                    nc.vector.tensor_tensor(out=scl2[:, sl, :], in0=mx2[:, sl, :],
                                            in1=mn2[:, sl, :], op=Alu.subtract)
                    nc.vector.tensor_scalar(out=scl2[:, sl, :], in0=scl2[:, sl, :],
                                            scalar1=INV15, scalar2=1e-8,
                                            op0=Alu.mult, op1=Alu.max)
                    nc.vector.reciprocal(out=rcp2[:, sl, :], in_=scl2[:, sl, :])
                    nc.vector.scalar_tensor_tensor(
                        out=zero2[:, sl, :], in0=mn2[:, sl, :], scalar=-1.0,
                        in1=rcp2[:, sl, :], op0=Alu.mult, op1=Alu.mult)

                if split_first:
                    # group 0: k first so ACT can start quantizing ASAP
                    for kv in (0, 1):
                        sl = slice(kv, kv + 1)
                        nc.vector.tensor_reduce(out=mx2[:, sl, :], in_=x2[:, sl, :, :],
                                                axis=AX.X, op=Alu.max)
                        nc.vector.tensor_reduce(out=mn2[:, sl, :], in_=x2[:, sl, :, :],
                                                axis=AX.X, op=Alu.min)
                        stats_chain(sl)
                else:
                    nc.vector.tensor_reduce(out=mx2[:, :, :], in_=x2[:, :, :, :],
                                            axis=AX.X, op=Alu.max)
                    nc.vector.tensor_reduce(out=mn2[:, :, :], in_=x2[:, :, :, :],
                                            axis=AX.X, op=Alu.min)
                    stats_chain(slice(0, 2))

                d.update(mx2=mx2, mn2=mn2, scl2=scl2, rcp2=rcp2, zero2=zero2)

                if S:
                    pkt2 = d["pkt2"]
                    lohi2 = lohip.tile([128, 2, CP, D], U8, tag="lohi2")
                    pk32 = pkt2[:, :, :, :].bitcast(U32)
                    nc.vector.tensor_scalar(out=lohi2[:, :, :, 0:D // 2].bitcast(U32),
                                            in0=pk32, scalar1=0x0F0F0F0F, scalar2=None,
                                            op0=Alu.bitwise_and)
                    nc.vector.tensor_scalar(out=lohi2[:, :, :, D // 2:D].bitcast(U32),
                                            in0=pk32, scalar1=4, scalar2=0x0F0F0F0F,
                                            op0=Alu.logical_shift_right,
                                            op1=Alu.bitwise_and)
                    d["lohi2"] = lohi2

            # ---- prologue ----
            load_group(0)
            load_group(1)
            if S:
                scp = persist.tile([128, 2 * PC], F32, tag="scp", name="scp")
                zpp = persist.tile([128, 2 * PC], F32, tag="zpp", name="zpp")
                for kv, t in enumerate(("k", "v")):
                    cs = slice(kv * PC, (kv + 1) * PC)
                    nc.sync.dma_start(
                        out=scp[:, cs].rearrange("p (b hh c) -> p b hh c", b=B, hh=HC),
                        in_=ins_sc[t][:, :, :].rearrange("b hh (p c) -> p b hh c", p=128))
                    nc.sync.dma_start(
                        out=zpp[:, cs].rearrange("p (b hh c) -> p b hh c", b=B, hh=HC),
                        in_=ins_zp[t][:, :, :].rearrange("b hh (p c) -> p b hh c", p=128))
            load_group(2)

            dve_lookahead(0, split_first=True)
            if S:
                nzs = persist.tile([128, 2 * PC], F32, tag="nzs", name="nzs")
                nc.vector.tensor_tensor(out=nzs[:, :], in0=zpp[:, :], in1=scp[:, :],
                                        op=Alu.mult)
                nc.vector.tensor_scalar(out=nzs[:, :], in0=nzs[:, :], scalar1=-1.0,
                                        scalar2=None, op0=Alu.mult)

            # ---- steady blocks ----
            def emit_stores_pre(j):
                dp = st[j]
                bp, hp = groups[j]
                if S:
                    for kv, t in enumerate(("k", "v")):
                        nc.sync.dma_start(
                            out=outs[t][bp, hp, 0:S, :].rearrange(
                                "(p c) d -> p c d", p=128),
                            in_=dp["opre"][kv][:, :, :])

            def emit_stores_oq(j):
                dp = st[j]
                bp, hp = groups[j]
                for kv, t in enumerate(("k", "v")):
                    nc.sync.dma_start(
                        out=outs[t][bp, hp, S:E, :].rearrange(
                            "(p c) d -> p c d", p=128),
                        in_=dp["oq"][kv][:, :, :])

            def emit_stores(j):
                emit_stores_pre(j)
                emit_stores_oq(j)

            def emit_deq(j):
                """Dequant of group j (q/stats written in block j)."""
                d = st[j]
                q, oq = d["q"], d["oq"]
                deq_a, deq_p = DEQ_SPLIT.get(j, DEQ_DEFAULT)

                def chunks(eng, kv, lo, hi):
                    for cc in range(lo, hi):
                        dst = oq[kv][:, cc, :]
                        if eng == "act":
                            nc.scalar.activation(out=dst,
                                                 in_=q[kv][:, cc, :],
                                                 func=Act.Identity,
                                                 bias=d["mn2"][:, kv, cc:cc + 1],
                                                 scale=d["scl2"][:, kv, cc:cc + 1])
                        elif eng == "gps":
                            nc.gpsimd.tensor_scalar(
                                out=dst, in0=q[kv][:, cc, :],
                                scalar1=d["scl2"][:, kv, cc:cc + 1],
                                scalar2=d["mn2"][:, kv, cc:cc + 1],
                                op0=Alu.mult, op1=Alu.add)
                        else:
                            nc.vector.tensor_scalar(
                                out=dst, in0=q[kv][:, cc, :],
                                scalar1=d["scl2"][:, kv, cc:cc + 1],
                                scalar2=d["mn2"][:, kv, cc:cc + 1],
                                op0=Alu.mult, op1=Alu.add)

                for kv in range(2):
                    chunks("act", kv, 0, deq_a)
                    chunks("gps", kv, deq_a, deq_a + deq_p)
                    chunks("dve", kv, deq_a + deq_p, CQ)

            for g in range(NG):
                b, hh = groups[g]
                d = st[g]

                # SP: loads first (buffer frees >= 1 block stale), then
                # stores of g-2 (data >= 1 block stale); eager at the tail
                if g + PREFETCH < NG:
                    load_group(g + PREFETCH)
                if 2 <= g <= 5:
                    emit_stores(g - 2)
                elif g >= 6:
                    emit_stores_pre(g - 1)

                # ACT: deq share of g-1 (stale), then quant k(g), v(g)
                if g > 0:
                    pass  # deq of g-1 is emitted below after lookahead so
                          # DVE stream order stays lookahead-first

                # DVE lookahead for g+1
                if g + 1 < NG:
                    dve_lookahead(g + 1)

                # deq of g-1 on all three engines (ACT before its quant via
                # emission order: emit_deq comes first for the ACT stream).
                # In the tail (no loads left behind them on SP), the freshest
                # ready stores follow their deq immediately.
                if g > 0:
                    emit_deq(g - 1)
                    if g == 5:
                        emit_stores(g - 1)
                    elif g >= 6:
                        emit_stores_oq(g - 1)

                # ACT: quant k then v (q per unit so deq deps are per-tensor)
                q = [qp.tile([128, CQ, D], U8, tag=f"q{kv}", name=f"q{kv}_{g}")
                     for kv in range(2)]
                for kv in range(2):
                    for cc in range(CQ):
                        nc.scalar.activation(out=q[kv][:, cc, :],
                                             in_=d["x2"][:, kv, cc, :],
                                             func=Act.Identity,
                                             bias=d["zero2"][:, kv, cc:cc + 1],
                                             scale=d["rcp2"][:, kv, cc:cc + 1])

                # Pool: prefix dequant of g (lohi2 ready one block ago)
                opre = None
                if S:
                    opre = [oprep.tile([128, CP, D], F32, tag=f"opre{kv}",
                                       name=f"opre{kv}_{g}") for kv in range(2)]
                    lohi2 = d["lohi2"]
                    pre_p = PRE_SPLIT.get(g, CP)
                    for kv in range(2):
                        pcol = (b * HC + hh) * CP + kv * PC
                        for cc in range(CP):
                            src = lohi2[:, kv, cc, :].rearrange(
                                "p (two dd) -> p two dd", two=2)
                            dst = opre[kv][:, cc, :].rearrange(
                                "p (dd two) -> p two dd", two=2)
                            pc = pcol + cc
                            if cc < pre_p:
                                nc.gpsimd.tensor_scalar(out=dst, in0=src,
                                                        scalar1=scp[:, pc:pc + 1],
                                                        scalar2=nzs[:, pc:pc + 1],
                                                        op0=Alu.mult, op1=Alu.add)
                            else:
                                nc.vector.tensor_scalar(out=dst, in0=src,
                                                        scalar1=scp[:, pc:pc + 1],
                                                        scalar2=nzs[:, pc:pc + 1],
                                                        op0=Alu.mult, op1=Alu.add)

                oq = [oqp.tile([128, CQ, D], F32, tag=f"oq{kv}",
                      name=f"oq{kv}_{g}") for kv in range(2)]
                d["q"] = q
                d["oq"] = oq
                d["opre"] = opre

            # ---- tail: opre(7) stores as soon as prefix(7) lands, then
            # dequant of group 7 and its oq stores ----
            emit_stores_pre(NG - 1)
            emit_deq(NG - 1)
            emit_stores_oq(NG - 1)

    if split_mw:
        _split_multiwait(nc)
    return nc


_CACHE = {}


def _get_nc(start_pos: int):
    if start_pos not in _CACHE:
        _CACHE[start_pos] = _build(start_pos)
    return _CACHE[start_pos]


def _install_ntff_hook_shim():
    """The agent image's antenv lacks axon_hooks; recreate it so
    run_bass_kernel_spmd(trace=True) can drive NTFF profiling."""
    import types
    if "antenv.axon_hooks" in sys.modules:
        return
    mod = types.ModuleType("antenv.axon_hooks")
    state = {"hook": None}
    try:
        from trn_agent_boot.trn_boot import _ntff_profile_via_ctypes
        state["hook"] = _ntff_profile_via_ctypes("/opt/axon/libaxon_pjrt.so")
    except Exception:
        pass
    mod.get_axon_ntff_profile_hook = lambda: state["hook"]
    mod.set_axon_ntff_profile_hook = lambda h: state.__setitem__("hook", h)
    sys.modules["antenv.axon_hooks"] = mod


def _kernel_np(k, v, k_cache, v_cache, k_scale, k_zero, v_scale, v_zero, start_pos):
    """Pure-numpy fallback for shapes the bass path doesn't handle."""
    def qp(x):
        mn = x.min(-1, keepdims=True)
        mx = x.max(-1, keepdims=True)
        scale = np.maximum((mx - mn) / np.float32(15.0), np.float32(1e-8))
        zero = -mn / scale
        q = np.clip(np.round(x / scale + zero), 0, 15).astype(np.uint8)
        return (q[..., 0::2] | (q[..., 1::2] << 4)), scale[..., 0], zero[..., 0]

    def dq(p, s, z):
        lo = (p & 15).astype(np.float32)
        hi = ((p >> 4) & 15).astype(np.float32)
        q = np.stack([lo, hi], -1).reshape(p.shape[:-1] + (p.shape[-1] * 2,))
        return (q - z[..., None]) * s[..., None]

    S = int(start_pos)
    E = S + k.shape[2]
    outs = []
    for x, cache, sc, zp in ((k, k_cache, k_scale, k_zero), (v, v_cache, v_scale, v_zero)):
        pp, ps, pz = qp(x)
        cache = cache.copy(); sc = sc.copy(); zp = zp.copy()
        cache[:, :, S:E] = pp
        sc[:, :, S:E] = ps
        zp[:, :, S:E] = pz
        outs.append(dq(cache[:, :, :E], sc[:, :, :E], zp[:, :, :E]))
    return tuple(outs)


def kernel(k, v, k_cache, v_cache, k_scale, k_zero, v_scale, v_zero, start_pos,
           _trace=False):
    k = np.asarray(k, np.float32)
    v = np.asarray(v, np.float32)
    k_cache = np.asarray(k_cache, np.uint8)
    v_cache = np.asarray(v_cache, np.uint8)
    k_scale = np.asarray(k_scale, np.float32)
    k_zero = np.asarray(k_zero, np.float32)
    v_scale = np.asarray(v_scale, np.float32)
    v_zero = np.asarray(v_zero, np.float32)
    S = int(start_pos)

    if (k.shape != (B, H, L, D) or S % 128 or S + L > MAX_SEQ):
        return _kernel_np(k, v, k_cache, v_cache, k_scale, k_zero, v_scale, v_zero, S)

    nc = _get_nc(S)
    E = S + L

    in_maps = []
    for m in range(N_CORES):
        hs = slice(m * HC, (m + 1) * HC)
        im = {
            "xk": np.ascontiguousarray(k[:, hs]),
            "xv": np.ascontiguousarray(v[:, hs]),
        }
        if S:
            im["pk"] = np.ascontiguousarray(k_cache[:, hs, :S, :])
            im["pv"] = np.ascontiguousarray(v_cache[:, hs, :S, :])
            im["sck"] = np.ascontiguousarray(k_scale[:, hs, :S])
            im["zpk"] = np.ascontiguousarray(k_zero[:, hs, :S])
            im["scv"] = np.ascontiguousarray(v_scale[:, hs, :S])
            im["zpv"] = np.ascontiguousarray(v_zero[:, hs, :S])
        in_maps.append(im)

    if _trace:
        _install_ntff_hook_shim()
    res = run_bass_kernel_spmd(nc, in_maps, list(range(N_CORES)), trace=_trace)

    k_dec = np.empty((B, H, E, D), np.float32)
    v_dec = np.empty((B, H, E, D), np.float32)
    for m in range(N_CORES):
        hs = slice(m * HC, (m + 1) * HC)
        k_dec[:, hs] = res.results[m]["ok"]
        v_dec[:, hs] = res.results[m]["ov"]
    if _trace:
        return (k_dec, v_dec), res
    return k_dec, v_dec


# revision 16
# speedup vs baseline: 1.0917x; 1.0917x over previous
"""CompressedKVCache kernel for Trainium2 (8 NeuronCores, head-sharded).

Per (b, h) head: quantize k/v rows to int4 (per-row min/max affine),
then return the dequantized cache prefix [0, start+L): rows [0, start)
decoded from the packed uint8 cache inputs, rows [start, start+L)
quantize->dequantized entirely on-chip.

Sharding: H=32 heads split across 8 cores (4 heads each); fully
independent per head, no cross-core communication.

v4 schedule: k and v of one (b, hh) form a GROUP (8 groups/core).
Shared x2/pkt2 tiles let the min/max reduces, the stats chain and the
nibble unpack run paired (one instruction covers both tensors), cutting
DVE instruction overhead.

Staleness discipline (the v2 lesson): every cross-engine dependency is
satisfied >= 1 block before its consumer issues, so no engine's wait
queue ever backs up and the in-order SP DMA queue never head-of-line
blocks the input loads. Per block g:
  SP   - loads of group g+3, then stores of group g-2 (all inputs to
         these DMAs were ready by the end of block g-1)
  DVE  - reduces/stats/unpack for g+1 (lookahead), then deq share of g-1
  ACT  - deq share of g-1 first, then quant k(g), quant v(g)
  Pool - prefix dequant of g (lohi from block g-1), deq share of g-1
Tail: from block 5 on SP has no loads left, so stores turn eager
(freshest-ready), leaving only group 7's dequant + 2 MiB of oq stores
after the last block.

Engine split per group (96 affine chunks of [128,128]):
  ACT 32 quant + 10 deq (~16.6us), Pool 32 prefix (~14.8us, the per-chunk
  slowest engine gets no deq in steady state), DVE reduces+stats+unpack
  (~12us) + 22 deq (~16.8us). The last two groups 3-way split their deq
  so the epilogue drains fast. All three engines RNE-round on f32->u8
  (probe-verified), but quant stays on ACT (fused scale/bias form).

Measured lessons (HW traces): any "just-stale" tile-ring reuse or
same-block cross-engine dependency adds ~100ns/instruction of semaphore
wait and can gate a consumer on the CUMULATIVE DMA semaphore (waiting for
unrelated later stores). Ring depths of 3+ and one-block staleness
everywhere are worth more than deeper prefetch or in-place-output tricks
(v5/v6 regressions).
"""

import sys

sys.path.insert(0, "/opt/trn_rl_repo")

import numpy as np
from concourse import bass, mybir
from concourse import tile
from concourse.bass_utils import run_bass_kernel_spmd

F32 = mybir.dt.float32
U8 = mybir.dt.uint8
U32 = mybir.dt.uint32
Alu = mybir.AluOpType
Act = mybir.ActivationFunctionType
AX = mybir.AxisListType
INV15 = float(np.float32(1.0 / 15.0))

B, H, L, D = 2, 32, 2048, 128
MAX_SEQ = 8192
N_CORES = 8
HC = H // N_CORES   # heads per core
CQ = L // 128       # quant row-chunks per head (16)
NG = B * HC         # groups per core (8); one group = k+v of one (b,hh)

# Per-unit deq chunk split (of CQ=16): (ACT, Pool) -- DVE gets the rest.
# Pool is the busiest engine (32 prefix chunks/group at ~460ns), so in
# steady state it gets no deq work; the last groups 3-way split so the
# epilogue dequant of group 7 finishes fast on all engines.
DEQ_SPLIT = {6: (0, 0), 7: (7, 3)}
DEQ_DEFAULT = (5, 0)
# Prefix chunks on Pool per unit for the tail group (DVE takes the rest)
PRE_SPLIT = {7: 12}

PREFETCH = 3  # groups of input lookahead


def _split_multiwait(nc):
    """This container's walrus accepts only ONE sync-wait per instruction;
    Tile's tail drain (and occasionally other insts) carry several. Split
    extras into single-wait EventSemaphore insts inserted just before."""
    for fn in nc.m.functions:
        for blk in fn.blocks:
            out = []
            for ins in blk.instructions:
                si = ins.sync_info
                if si is not None and si.on_wait is not None and len(si.on_wait) > 1:
                    waits = list(si.on_wait)
                    for j, w in enumerate(waits[:-1]):
                        out.append(mybir.InstEventSemaphore(
                            name=f"{ins.name}_sw{j}", ins=[], outs=[],
                            engine=ins.engine,
                            sync_info=mybir.SyncInfo(on_wait=[w], on_update=[])))
                    si.on_wait = [waits[-1]]
                    ins.sync_info = si
                out.append(ins)
            blk.instructions = out


def _build(start_pos: int, split_mw: bool = True):
    """Per core: xk/xv (B,HC,L,D) f32, prefix packed caches (B,HC,S,64) u8
    and prefix scale/zero rows (B,HC,S) f32 -> ok/ov (B,HC,S+L,D) f32."""
    S = start_pos
    E = S + L
    CP = S // 128  # prefix row-chunks per head
    assert L % 128 == 0 and S % 128 == 0 and E <= MAX_SEQ

    nc = bass.Bass(trn_type="TRN2")

    ins_q, ins_p, ins_sc, ins_zp, outs = {}, {}, {}, {}, {}
    for t in ("k", "v"):
        ins_q[t] = nc.dram_tensor(f"x{t}", [B, HC, L, D], F32, kind="ExternalInput")
        if S:
            ins_p[t] = nc.dram_tensor(f"p{t}", [B, HC, S, D // 2], U8, kind="ExternalInput")
            ins_sc[t] = nc.dram_tensor(f"sc{t}", [B, HC, S], F32, kind="ExternalInput")
            ins_zp[t] = nc.dram_tensor(f"zp{t}", [B, HC, S], F32, kind="ExternalInput")
        outs[t] = nc.dram_tensor(f"o{t}", [B, HC, E, D], F32, kind="ExternalOutput")

    groups = [(b, hh) for b in range(B) for hh in range(HC)]
    PC = B * HC * CP  # hoisted scale/zero columns per tensor

    with tile.TileContext(nc) as tc:
        with tc.tile_pool(name="xp", bufs=PREFETCH + 1) as xp, \
             tc.tile_pool(name="pktp", bufs=PREFETCH + 1) as pktp, \
             tc.tile_pool(name="lohip", bufs=3) as lohip, \
             tc.tile_pool(name="qp", bufs=3) as qp, \
             tc.tile_pool(name="oqp", bufs=3) as oqp, \
             tc.tile_pool(name="oprep", bufs=3) as oprep, \
             tc.tile_pool(name="small", bufs=4) as small, \
             tc.tile_pool(name="persist", bufs=1) as persist:

            st = [None] * NG  # live tiles per group

            def load_group(g):
                b, hh = groups[g]
                x2 = xp.tile([128, 2, CQ, D], F32, tag="x2")
                for kv, t in enumerate(("k", "v")):
                    nc.sync.dma_start(
                        out=x2[:, kv, :, :],
                        in_=ins_q[t][b, hh, :, :].rearrange("(p c) d -> p c d", p=128))
                pkt2 = None
                if S:
                    pkt2 = pktp.tile([128, 2, CP, D // 2], U8, tag="pkt2")
                    for kv, t in enumerate(("k", "v")):
                        nc.sync.dma_start(
                            out=pkt2[:, kv, :, :],
                            in_=ins_p[t][b, hh, :, :].rearrange("(p c) d -> p c d", p=128))
                st[g] = {"x2": x2, "pkt2": pkt2}

            def dve_lookahead(g, split_first=False):
                """Reduces + stats (+ unpack) for group g on DVE."""
                d = st[g]
                x2 = d["x2"]
                mx2 = small.tile([128, 2, CQ], F32, tag="mx2")
                mn2 = small.tile([128, 2, CQ], F32, tag="mn2")
                scl2 = small.tile([128, 2, CQ], F32, tag="scl2")
                rcp2 = small.tile([128, 2, CQ], F32, tag="rcp2")
                zero2 = small.tile([128, 2, CQ], F32, tag="zero2")

                def stats_chain(sl):
                    nc.vector.tensor_tensor(out=scl2[:, sl, :], in0=mx2[:, sl, :],
                                            in1=mn2[:, sl, :], op=Alu.subtract)
                    nc.vector.tensor_scalar(out=scl2[:, sl, :], in0=scl2[:, sl, :],
                                            scalar1=INV15, scalar2=1e-8,
                                            op0=Alu.mult, op1=Alu.max)
                    nc.vector.reciprocal(out=rcp2[:, sl, :], in_=scl2[:, sl, :])
                    nc.vector.scalar_tensor_tensor(
                        out=zero2[:, sl, :], in0=mn2[:, sl, :], scalar=-1.0,
                        in1=rcp2[:, sl, :], op0=Alu.mult, op1=Alu.mult)

                if split_first:
                    # group 0: k first so ACT can start quantizing ASAP
                    for kv in (0, 1):
                        sl = slice(kv, kv + 1)
                        nc.vector.tensor_reduce(out=mx2[:, sl, :], in_=x2[:, sl, :, :],
                                                axis=AX.X, op=Alu.max)
                        nc.vector.tensor_reduce(out=mn2[:, sl, :], in_=x2[:, sl, :, :],
                                                axis=AX.X, op=Alu.min)
                        stats_chain(sl)
                else:
                    nc.vector.tensor_reduce(out=mx2[:, :, :], in_=x2[:, :, :, :],
                                            axis=AX.X, op=Alu.max)
                    nc.vector.tensor_reduce(out=mn2[:, :, :], in_=x2[:, :, :, :],
                                            axis=AX.X, op=Alu.min)
                    stats_chain(slice(0, 2))

                d.update(mx2=mx2, mn2=mn2, scl2=scl2, rcp2=rcp2, zero2=zero2)

                if S:
                    pkt2 = d["pkt2"]
                    lohi2 = lohip.tile([128, 2, CP, D], U8, tag="lohi2")
                    pk32 = pkt2[:, :, :, :].bitcast(U32)
                    nc.vector.tensor_scalar(out=lohi2[:, :, :, 0:D // 2].bitcast(U32),
                                            in0=pk32, scalar1=0x0F0F0F0F, scalar2=None,
                                            op0=Alu.bitwise_and)
                    nc.vector.tensor_scalar(out=lohi2[:, :, :, D // 2:D].bitcast(U32),
                                            in0=pk32, scalar1=4, scalar2=0x0F0F0F0F,
                                            op0=Alu.logical_shift_right,
                                            op1=Alu.bitwise_and)
                    d["lohi2"] = lohi2

            # ---- prologue ----
            load_group(0)
            load_group(1)
            if S:
                scp = persist.tile([128, 2 * PC], F32, tag="scp", name="scp")
                zpp = persist.tile([128, 2 * PC], F32, tag="zpp", name="zpp")
                for kv, t in enumerate(("k", "v")):
                    cs = slice(kv * PC, (kv + 1) * PC)
                    nc.sync.dma_start(
                        out=scp[:, cs].rearrange("p (b hh c) -> p b hh c", b=B, hh=HC),
                        in_=ins_sc[t][:, :, :].rearrange("b hh (p c) -> p b hh c", p=128))
                    nc.sync.dma_start(
                        out=zpp[:, cs].rearrange("p (b hh c) -> p b hh c", b=B, hh=HC),
                        in_=ins_zp[t][:, :, :].rearrange("b hh (p c) -> p b hh c", p=128))
            load_group(2)

            dve_lookahead(0, split_first=True)
            if S:
                nzs = persist.tile([128, 2 * PC], F32, tag="nzs", name="nzs")
                nc.vector.tensor_tensor(out=nzs[:, :], in0=zpp[:, :], in1=scp[:, :],
                                        op=Alu.mult)
                nc.vector.tensor_scalar(out=nzs[:, :], in0=nzs[:, :], scalar1=-1.0,
                                        scalar2=None, op0=Alu.mult)

            # ---- steady blocks ----
            def emit_stores_pre(j):
                dp = st[j]
                bp, hp = groups[j]
                if S:
                    for kv, t in enumerate(("k", "v")):
                        nc.sync.dma_start(
                            out=outs[t][bp, hp, 0:S, :].rearrange(
                                "(p c) d -> p c d", p=128),
                            in_=dp["opre"][kv][:, :, :])

            def emit_stores_oq(j):
                dp = st[j]
                bp, hp = groups[j]
                for kv, t in enumerate(("k", "v")):
                    nc.sync.dma_start(
                        out=outs[t][bp, hp, S:E, :].rearrange(
                            "(p c) d -> p c d", p=128),
                        in_=dp["oq"][kv][:, :, :])

            def emit_stores(j):
                emit_stores_pre(j)
                emit_stores_oq(j)

            def emit_deq(j):
                """Dequant of group j (q/stats written in block j)."""
                d = st[j]
                q, oq = d["q"], d["oq"]
                deq_a, deq_p = DEQ_SPLIT.get(j, DEQ_DEFAULT)

                def chunks(eng, kv, lo, hi):
                    for cc in range(lo, hi):
                        dst = oq[kv][:, cc, :]
                        if eng == "act":
                            nc.scalar.activation(out=dst,
                                                 in_=q[kv][:, cc, :],
                                                 func=Act.Identity,
                                                 bias=d["mn2"][:, kv, cc:cc + 1],
                                                 scale=d["scl2"][:, kv, cc:cc + 1])
                        elif eng == "gps":
                            nc.gpsimd.tensor_scalar(
                                out=dst, in0=q[kv][:, cc, :],
                                scalar1=d["scl2"][:, kv, cc:cc + 1],
                                scalar2=d["mn2"][:, kv, cc:cc + 1],
                                op0=Alu.mult, op1=Alu.add)
                        else:
                            nc.vector.tensor_scalar(
                                out=dst, in0=q[kv][:, cc, :],
                                scalar1=d["scl2"][:, kv, cc:cc + 1],
                                scalar2=d["mn2"][:, kv, cc:cc + 1],
                                op0=Alu.mult, op1=Alu.add)

                for kv in range(2):
                    chunks("act", kv, 0, deq_a)
                    chunks("gps", kv, deq_a, deq_a + deq_p)
                    chunks("dve", kv, deq_a + deq_p, CQ)

            for g in range(NG):
                b, hh = groups[g]
                d = st[g]

                # SP: loads first (buffer frees >= 1 block stale), then
                # stores of g-2 (data >= 1 block stale); eager at the tail
                if g + PREFETCH < NG:
                    load_group(g + PREFETCH)
                if 2 <= g <= 5:
                    emit_stores(g - 2)
                elif g >= 6:
                    emit_stores_pre(g - 1)

                # ACT: deq share of g-1 (stale), then quant k(g), v(g)
                if g > 0:
                    pass  # deq of g-1 is emitted below after lookahead so
                          # DVE stream order stays lookahead-first

                # DVE lookahead for g+1
                if g + 1 < NG:
                    dve_lookahead(g + 1)

                # deq of g-1 on all three engines (ACT before its quant via
                # emission order: emit_deq comes first for the ACT stream).
                # In the tail (no loads left behind them on SP), the freshest
                # ready stores follow their deq immediately.
                if g > 0:
                    emit_deq(g - 1)
                    if g == 5:
                        emit_stores(g - 1)
                    elif g >= 6:
                        emit_stores_oq(g - 1)

                # ACT: quant k then v (q per unit so deq deps are per-tensor)
                q = [qp.tile([128, CQ, D], U8, tag=f"q{kv}", name=f"q{kv}_{g}")
                     for kv in range(2)]
                for kv in range(2):
                    for cc in range(CQ):
                        nc.scalar.activation(out=q[kv][:, cc, :],
                                             in_=d["x2"][:, kv, cc, :],
                                             func=Act.Identity,
                                             bias=d["zero2"][:, kv, cc:cc + 1],
                                             scale=d["rcp2"][:, kv, cc:cc + 1])

                # Pool: prefix dequant of g (lohi2 ready one block ago)
                opre = None
                if S:
                    opre = [oprep.tile([128, CP, D], F32, tag=f"opre{kv}",
                                       name=f"opre{kv}_{g}") for kv in range(2)]
                    lohi2 = d["lohi2"]
                    pre_p = PRE_SPLIT.get(g, CP)
                    for kv in range(2):
                        pcol = (b * HC + hh) * CP + kv * PC
                        for cc in range(CP):
                            src = lohi2[:, kv, cc, :].rearrange(
                                "p (two dd) -> p two dd", two=2)
                            dst = opre[kv][:, cc, :].rearrange(
                                "p (dd two) -> p two dd", two=2)
                            pc = pcol + cc
                            if cc < pre_p:
                                nc.gpsimd.tensor_scalar(out=dst, in0=src,
                                                        scalar1=scp[:, pc:pc + 1],
                                                        scalar2=nzs[:, pc:pc + 1],
                                                        op0=Alu.mult, op1=Alu.add)
                            else:
                                nc.vector.tensor_scalar(out=dst, in0=src,
                                                        scalar1=scp[:, pc:pc + 1],
                                                        scalar2=nzs[:, pc:pc + 1],
                                                        op0=Alu.mult, op1=Alu.add)

                oq = [oqp.tile([128, CQ, D], F32, tag=f"oq{kv}",
                      name=f"oq{kv}_{g}") for kv in range(2)]
                d["q"] = q
                d["oq"] = oq
                d["opre"] = opre

            # ---- tail: opre(7) stores as soon as prefix(7) lands, then
            # dequant of group 7 and its oq stores ----
            emit_stores_pre(NG - 1)
            emit_deq(NG - 1)
            emit_stores_oq(NG - 1)

    if split_mw:
        _split_multiwait(nc)
    return nc


_CACHE = {}


def _get_nc(start_pos: int):
    if start_pos not in _CACHE:
        _CACHE[start_pos] = _build(start_pos)
    return _CACHE[start_pos]


def _install_ntff_hook_shim():
    """The agent image's antenv lacks axon_hooks; recreate it so
    run_bass_kernel_spmd(trace=True) can drive NTFF profiling."""
    import types
    if "antenv.axon_hooks" in sys.modules:
        return
    mod = types.ModuleType("antenv.axon_hooks")
    state = {"hook": None}
    try:
        from trn_agent_boot.trn_boot import _ntff_profile_via_ctypes
        state["hook"] = _ntff_profile_via_ctypes("/opt/axon/libaxon_pjrt.so")
    except Exception:
        pass
    mod.get_axon_ntff_profile_hook = lambda: state["hook"]
    mod.set_axon_ntff_profile_hook = lambda h: state.__setitem__("hook", h)
    sys.modules["antenv.axon_hooks"] = mod


def _kernel_np(k, v, k_cache, v_cache, k_scale, k_zero, v_scale, v_zero, start_pos):
    """Pure-numpy fallback for shapes the bass path doesn't handle."""
    def qp(x):
        mn = x.min(-1, keepdims=True)
        mx = x.max(-1, keepdims=True)
        scale = np.maximum((mx - mn) / np.float32(15.0), np.float32(1e-8))
        zero = -mn / scale
        q = np.clip(np.round(x / scale + zero), 0, 15).astype(np.uint8)
        return (q[..., 0::2] | (q[..., 1::2] << 4)), scale[..., 0], zero[..., 0]

    def dq(p, s, z):
        lo = (p & 15).astype(np.float32)
        hi = ((p >> 4) & 15).astype(np.float32)
        q = np.stack([lo, hi], -1).reshape(p.shape[:-1] + (p.shape[-1] * 2,))
        return (q - z[..., None]) * s[..., None]

    S = int(start_pos)
    E = S + k.shape[2]
    outs = []
    for x, cache, sc, zp in ((k, k_cache, k_scale, k_zero), (v, v_cache, v_scale, v_zero)):
        pp, ps, pz = qp(x)
        cache = cache.copy(); sc = sc.copy(); zp = zp.copy()
        cache[:, :, S:E] = pp
        sc[:, :, S:E] = ps
        zp[:, :, S:E] = pz
        outs.append(dq(cache[:, :, :E], sc[:, :, :E], zp[:, :, :E]))
    return tuple(outs)


def kernel(k, v, k_cache, v_cache, k_scale, k_zero, v_scale, v_zero, start_pos,
           _trace=False):
    k = np.asarray(k, np.float32)
    v = np.asarray(v, np.float32)
    k_cache = np.asarray(k_cache, np.uint8)
    v_cache = np.asarray(v_cache, np.uint8)
    k_scale = np.asarray(k_scale, np.float32)
    k_zero = np.asarray(k_zero, np.float32)
    v_scale = np.asarray(v_scale, np.float32)
    v_zero = np.asarray(v_zero, np.float32)
    S = int(start_pos)

    if (k.shape != (B, H, L, D) or S % 128 or S + L > MAX_SEQ):
        return _kernel_np(k, v, k_cache, v_cache, k_scale, k_zero, v_scale, v_zero, S)

    nc = _get_nc(S)
    E = S + L

    in_maps = []
    for m in range(N_CORES):
        hs = slice(m * HC, (m + 1) * HC)
        im = {
            "xk": np.ascontiguousarray(k[:, hs]),
            "xv": np.ascontiguousarray(v[:, hs]),
        }
        if S:
            im["pk"] = np.ascontiguousarray(k_cache[:, hs, :S, :])
            im["pv"] = np.ascontiguousarray(v_cache[:, hs, :S, :])
            im["sck"] = np.ascontiguousarray(k_scale[:, hs, :S])
            im["zpk"] = np.ascontiguousarray(k_zero[:, hs, :S])
            im["scv"] = np.ascontiguousarray(v_scale[:, hs, :S])
            im["zpv"] = np.ascontiguousarray(v_zero[:, hs, :S])
        in_maps.append(im)

    if _trace:
        _install_ntff_hook_shim()
    res = run_bass_kernel_spmd(nc, in_maps, list(range(N_CORES)), trace=_trace)

    k_dec = np.empty((B, H, E, D), np.float32)
    v_dec = np.empty((B, H, E, D), np.float32)
    for m in range(N_CORES):
        hs = slice(m * HC, (m + 1) * HC)
        k_dec[:, hs] = res.results[m]["ok"]
        v_dec[:, hs] = res.results[m]["ov"]
    if _trace:
        return (k_dec, v_dec), res
    return k_dec, v_dec
